# revision 25
# baseline (speedup 1.0000x reference)
"""FPN ROI-Align pooler — v3: plain-table row-granular gathers on 8 TRN2 cores.

Table is plain channels-last [106250+pad, 256] fp8e3 (one entry per pixel).
Each bin decomposes into slots = (distinct y-row) x (x-window); a slot
fetches nT consecutive pixels (near: K cols of one row, nT=K; far bins use
two 2-col windows, nT=2). Distinct-row dedup drops the redundant 4th y-row
when the two y-samples' row pairs overlap (~45% of bins), cutting gather
bytes and PE work ~13% vs the banded v2.

Classes are (nT, spb) with spb = slots/bin in {2,3,4,6,8}; ts = 128//spb
bins/tile. Per class the distinct fetch blocks are compacted into a DRAM
block table so ids fit dma_gather's int16 indices (wide classes split into
windowed groups). PSUM banks [128, 512] hold 2 column-stacks of k
row-groups (k=2 at bases {0,64} for ts>32, else k=4 at {0,32,64,96}); one
ACT copy drains a bank and one HWDGE store flushes a batch into
stack-padded DRAM rows (dead rows masked via slotmap).
"""

import sys

import numpy as np

if "/opt/trn_rl_repo" not in sys.path:
    sys.path.insert(0, "/opt/trn_rl_repo")

OUT = 7
SR = 2
SCALES = (0.25, 0.125, 0.0625, 0.03125)
K_MIN = 2
CANON_SCALE = 224.0
CANON_LVL = 4.0
EPS = 1e-6

B, C, N = 2, 256, 1000
SIZES = ((200, 200), (100, 100), (50, 50), (25, 25))
NCORES = 8
NBIN = OUT * OUT
MAXSPB = 8
MAXNT = 4
WINCAP = 32700

LEVEL_BASE = []
_acc = 0
for _h, _w in SIZES:
    LEVEL_BASE.append(_acc)
    _acc += B * _h * _w
TOTAL_ROWS = _acc  # 106250 pixels
PAD_ROWS = 8
TBL_ROWS = TOTAL_ROWS + PAD_ROWS

TABLE_DT = "float8e3"

CLASSES = ((2, 2), (2, 3), (2, 4), (2, 6), (2, 8),
           (3, 2), (3, 3), (3, 4), (4, 2), (4, 3), (4, 4))
CLS_BY = {c: i for i, c in enumerate(CLASSES)}
WBATCH = (8, 8, 8, 8, 8, 6, 6, 6, 4, 4, 4)

_PROGRAM_CACHE: dict = {}


def _np_table_dt(name):
    if name == "float8e3":
        import ml_dtypes
        return ml_dtypes.float8_e3m4
    return {"float16": np.float16, "float32": np.float32}[name]


def _ts_of(ci):
    return 128 // CLASSES[ci][1]


def _k_of(ci):
    ts = _ts_of(ci)
    if ts >= 128:
        return 1
    return 2 if ts > 32 else 4


def _pb_step(ci):
    return 64 if _k_of(ci) == 2 else 32


def _axis_precompute(lo, hi, Wdim):
    f32 = np.float32
    roi = np.maximum(hi - lo, f32(1.0))
    bin_sz = roi / f32(OUT)
    a_out = np.arange(OUT, dtype=f32)[None, :, None]
    a_sr = np.arange(SR, dtype=f32)[None, None, :]
    grid = a_out * bin_sz[:, None, None] + (a_sr + f32(0.5)) * bin_sz[:, None, None] / f32(SR)
    pos = (lo[:, None, None] + grid).reshape(N, OUT * SR)
    Wf = Wdim.astype(f32)
    valid = (pos >= f32(-1.0)) & (pos <= Wf[:, None])
    p = np.clip(pos, f32(0.0), (Wf - f32(1.0))[:, None])
    p0f = np.floor(p)
    p0 = p0f.astype(np.int32)
    p1 = np.minimum(p0 + 1, Wdim[:, None] - 1)
    lp = p - p0f
    hp = f32(1.0) - lp
    corn = np.stack([p0, p1], axis=-1)
    wgt = np.stack([hp, lp], axis=-1).astype(f32)
    return corn, wgt, valid


def _host_precompute(boxes, img_ids):
    """Per-bin slot decomposition. Returns cls [Nb], idxP [Nb, MAXSPB] plain
    pixel starts, wcP [Nb, MAXSPB, MAXNT] f32."""
    f32 = np.float32
    boxes = np.asarray(boxes, f32)
    x1, y1, x2, y2 = boxes[:, 0], boxes[:, 1], boxes[:, 2], boxes[:, 3]
    area = (x2 - x1) * (y2 - y1)
    s = np.sqrt(area)
    lvl = np.floor(f32(CANON_LVL) + np.log2(s / f32(CANON_SCALE) + f32(EPS)))
    lvl = np.clip(lvl, K_MIN, K_MIN + len(SCALES) - 1).astype(np.int32) - K_MIN

    scale = np.asarray(SCALES, f32)[lvl]
    Hs = np.asarray([h for h, w in SIZES], np.int32)[lvl]
    Ws = np.asarray([w for h, w in SIZES], np.int32)[lvl]
    base = np.asarray(LEVEL_BASE, np.int64)[lvl]

    xcorn, xw, xval = _axis_precompute(x1 * scale, x2 * scale, Ws)
    ycorn, yw, yval = _axis_precompute(y1 * scale, y2 * scale, Hs)
    ix = (np.arange(OUT)[:, None] * SR + np.arange(SR)[None, :])  # [7,2]

    # x side: near = one K-col window, far = two 2-col windows
    x0a = xcorn[:, ix[:, 0], 0]
    dcorn = xcorn[:, ix, :] - x0a[:, :, None, None]
    assert dcorn.min() >= 0
    dmax = dcorn.max(axis=(2, 3))
    kcls = np.zeros((N, OUT), np.int32)
    kcls[dmax > 1] = 1
    kcls[dmax > 2] = 2
    kcls[dmax > 3] = 3  # far

    wxc = (xw[:, ix, :] * xval[:, ix][:, :, :, None]) * f32(0.5)
    wx_pix = np.zeros((N, OUT, MAXNT), f32)
    n_i, b_i = np.meshgrid(np.arange(N), np.arange(OUT), indexing="ij")
    for kx in range(SR):
        for cx in range(2):
            d = np.minimum(dcorn[:, :, kx, cx], MAXNT - 1)
            np.add.at(wx_pix, (n_i, b_i, d), wxc[:, :, kx, cx])
    x0s = xcorn[:, ix, 0]
    weq = (xcorn[:, ix, 1] == xcorn[:, ix, 0])
    wfar = np.zeros((N, OUT, 2, 2), f32)
    wfar[..., 0] = wxc[..., 0] + wxc[..., 1] * weq
    wfar[..., 1] = wxc[..., 1] * (~weq)

    # y side: distinct rows of the 4 corner-rows, merged weights
    wyr = (yw[:, ix, :] * yval[:, ix][:, :, :, None]) * f32(0.5)
    cand_r = np.stack([ycorn[:, ix[:, 0], 0], ycorn[:, ix[:, 0], 1],
                       ycorn[:, ix[:, 1], 0], ycorn[:, ix[:, 1], 1]], axis=-1)
    cand_w = np.stack([wyr[:, :, 0, 0], wyr[:, :, 0, 1],
                       wyr[:, :, 1, 0], wyr[:, :, 1, 1]], axis=-1)
    order = np.argsort(cand_r, axis=-1, kind="stable")
    r_s = np.take_along_axis(cand_r, order, -1)
    w_s = np.take_along_axis(cand_w, order, -1)
    yrows = np.zeros((N, OUT, 4), np.int32)
    ywt = np.zeros((N, OUT, 4), f32)
    ycnt = np.zeros((N, OUT), np.int64)
    for j in range(4):
        r_j = r_s[..., j]
        w_j = w_s[..., j]
        is_new = np.ones(r_j.shape, bool) if j == 0 else (r_j != r_s[..., j - 1])
        ycnt = ycnt + is_new
        pos = (ycnt - 1)[..., None]
        np.put_along_axis(yrows, pos, r_j[..., None], axis=-1)
        cur = np.take_along_axis(ywt, pos, axis=-1)[..., 0]
        np.put_along_axis(ywt, pos, (cur + w_j)[..., None], axis=-1)
    keep = ywt != 0
    ycnt2 = keep.sum(-1).astype(np.int64)
    ordk = np.argsort(~keep, axis=-1, kind="stable")
    yrows = np.take_along_axis(yrows, ordk, -1)
    ywt = np.take_along_axis(ywt, ordk, -1)
    dead = ycnt2 == 0
    ycnt2[dead] = 1
    yrows[dead, 0] = 0
    ywt[dead, 0] = 0.0
    one = ycnt2 == 1   # fold spb-1 bins up to 2 slots (zero-weight dup)
    yrows[one, 1] = yrows[one, 0]
    ywt[one, 1] = 0.0
    ycnt2[one] = 2

    Nb = N * NBIN
    cls = np.zeros((N, OUT, OUT), np.int32)
    idxP = np.zeros((N, OUT, OUT, MAXSPB), np.int64)
    wcP = np.zeros((N, OUT, OUT, MAXSPB, MAXNT), f32)

    rowbase = (base[:, None, None] +
               (np.asarray(img_ids).astype(np.int64)[:, None, None] *
                Hs[:, None, None].astype(np.int64) +
                yrows.astype(np.int64)) * Ws[:, None, None].astype(np.int64))

    KXB = np.repeat(kcls[:, None, :], OUT, axis=1)
    YC = np.repeat(ycnt2[:, :, None], OUT, axis=2)
    for yc in (2, 3, 4):
        for ki, Kv in ((0, 2), (1, 3), (2, 4)):
            m = (KXB == ki) & (YC == yc)
            if not m.any():
                continue
            cls[m] = CLS_BY[(Kv, yc)]
            nn, bb, xx = np.where(m)
            for i in range(yc):
                idxP[nn, bb, xx, i] = rowbase[nn, bb, i] + x0a[nn, xx]
                wcP[nn, bb, xx, i, :Kv] = (ywt[nn, bb, i, None]
                                           * wx_pix[nn, xx, :Kv])
        m = (KXB == 3) & (YC == yc)
        if not m.any():
            continue
        cls[m] = CLS_BY[(2, 2 * yc)]
        nn, bb, xx = np.where(m)
        for i in range(yc):
            for xs in range(2):
                sl = i * 2 + xs
                idxP[nn, bb, xx, sl] = rowbase[nn, bb, i] + x0s[nn, xx, xs]
                wcP[nn, bb, xx, sl, :2] = (ywt[nn, bb, i, None]
                                           * wfar[nn, xx, xs])
    assert idxP.min() >= 0 and idxP.max() < TOTAL_ROWS
    return cls.reshape(Nb), idxP.reshape(Nb, MAXSPB).astype(np.int32), \
        np.ascontiguousarray(wcP.reshape(Nb, MAXSPB, MAXNT), dtype=f32)


def _make_table(feats, dtype):
    """Plain channels-last table [TBL_ROWS, C]."""
    parts = []
    for f in feats:
        _, _, H, W = f.shape
        nhwc = np.ascontiguousarray(f.transpose(0, 2, 3, 1))
        parts.append(nhwc.reshape(-1, C))
    parts.append(np.zeros((PAD_ROWS, C), parts[0].dtype))
    return np.ascontiguousarray(np.concatenate(parts, axis=0)).astype(dtype)


def _plan(cls, idxP):
    """Window-group bins per class for int16 block ids; pad groups to
    ts*NCORES bins. Returns plans, tcounts, blkmaps, blkrefs, segs."""
    plans, tcounts, blkmaps, blkrefs, segs = [], [], [], [], []
    for ci, (nT, spb) in enumerate(CLASSES):
        ts = _ts_of(ci)
        gran = ts * NCORES
        ids = np.where(cls == ci)[0]
        starts = idxP[ids][:, :spb]
        blocks = np.unique(starts)
        bid = np.searchsorted(blocks, starts)
        order = np.argsort(bid.min(axis=1), kind="stable")
        ids_s, bid_s = ids[order], bid[order]
        bmin = bid_s.min(axis=1)
        bmax = bid_s.max(axis=1)
        cuts = [0]
        gmin = bmin[0] if len(ids) else 0
        for i in range(1, len(ids)):
            if bmax[i] - gmin > WINCAP:
                cuts.append(i)
                gmin = bmin[i]
        cuts.append(len(ids))
        gids, gref, gbase, gtiles = [], [], [], []
        for gi in range(len(cuts) - 1):
            lo, hi = cuts[gi], cuts[gi + 1]
            n = hi - lo
            npad = int(np.ceil(n / gran)) * gran
            gi_ids = -np.ones(npad, np.int64)
            gi_ids[:n] = ids_s[lo:hi]
            bb = int(bmin[lo]) if n else 0
            gi_ref = np.zeros((npad, spb), np.int64)
            gi_ref[:n] = bid_s[lo:hi] - bb
            assert gi_ref.min() >= 0 and gi_ref.max() <= 32767
            gids.append(gi_ids.reshape(-1, ts))
            gref.append(gi_ref.reshape(-1, ts, spb))
            gbase.append(bb)
            gtiles.append(npad // ts)
        if not gids:
            gids = [np.full((0, ts), -1, np.int64)]
            gref = [np.zeros((0, ts, spb), np.int64)]
            gbase, gtiles = [0], [0]
        plans.append(np.concatenate(gids, axis=0))
        blkrefs.append(np.concatenate(gref, axis=0))
        blkmaps.append(blocks)
        tcounts.append(plans[-1].shape[0] // NCORES)
        seg, t0 = [], 0
        for g, nt_g in enumerate(gtiles):
            nt = nt_g // NCORES
            if nt:
                seg.append((t0, nt, gbase[g]))
            t0 += nt
        segs.append(seg)
    return plans, tuple(tcounts), blkmaps, blkrefs, segs


def _make_block_tables(tbl, blkmaps):
    """Per-class compacted block tables [n_blocks, nT*C] from the plain tbl."""
    ctbls = []
    for ci, (nT, spb) in enumerate(CLASSES):
        blocks = blkmaps[ci]
        if len(blocks) == 0:
            ctbls.append(np.zeros((1, nT * C), tbl.dtype))
            continue
        gath = tbl[blocks[:, None] + np.arange(nT)[None, :]]
        ctbls.append(np.ascontiguousarray(gath.reshape(len(blocks), nT * C)))
    return ctbls


def _class_geometry(tcounts, segs):
    """Per-class batches (t0, Wb, base, stk0) and stack-padded row extents."""
    geo = []
    r_off = 0
    for ci in range(len(CLASSES)):
        k = _k_of(ci)
        W = WBATCH[ci]
        batches = []
        stk = 0
        for (s_t0, s_nt, s_base) in segs[ci]:
            for t0 in range(s_t0, s_t0 + s_nt, W):
                Wb = min(W, s_t0 + s_nt - t0)
                batches.append((t0, Wb, s_base, stk))
                stk += (Wb + k - 1) // k
        geo.append({"r_off": r_off, "batches": batches, "nstacks": stk})
        r_off += stk * 128
    return geo, r_off


def _pack_core(core, plans, tcounts, blkrefs, wcP, geo, out_rows):
    T = sum(tcounts)
    idx16 = np.zeros((128, 8 * T), np.int16)
    wc_cols = sum(CLASSES[ci][0] * tcounts[ci] for ci in range(len(CLASSES)))
    wc_arr = np.zeros((128, wc_cols), np.float16)
    slotmap = np.full((out_rows,), -1, np.int64)

    t_off = 0
    c_off = 0
    p_ar = np.arange(128)
    for ci, (nT, spb) in enumerate(CLASSES):
        Tk = tcounts[ci]
        if Tk == 0:
            continue
        ts = _ts_of(ci)
        used = ts * spb
        k = _k_of(ci)
        pbs = _pb_step(ci)
        tiles = plans[ci][core::NCORES]       # [Tk, ts]
        refs = blkrefs[ci][core::NCORES]      # [Tk, ts, spb]
        valid = tiles >= 0
        ids = np.where(valid, tiles, 0)
        blk = np.zeros((Tk, 128), np.int64)
        blk[:, :used] = (refs * valid[:, :, None]).reshape(Tk, used)
        wrap = blk.T                           # [128, Tk]
        for t in range(Tk):
            col = (t_off + t) * 8 + (p_ar // 16)
            row = p_ar % 16
            for kk in range(8):
                idx16[16 * kk + row, col] = wrap[:, t]
        wv = wcP[ids][:, :, :spb, :nT] * valid[:, :, None, None]
        wv = wv.reshape(Tk, used, nT).transpose(1, 0, 2).reshape(used, Tk * nT)
        wc_arr[:used, c_off:c_off + Tk * nT] = wv.astype(np.float16)
        g = geo[ci]
        for (t0, Wb, _bse, stk0) in g["batches"]:
            for wi in range(Wb):
                t = t0 + wi
                s = stk0 + wi // k
                rg = wi % k
                rows = g["r_off"] + s * 128 + rg * pbs + np.arange(ts)
                slotmap[rows] = np.where(valid[t], tiles[t], -1)
        t_off += Tk
        c_off += Tk * nT
    return idx16, wc_arr, slotmap


def _tsp_of(spb):
    """Wall row count = psum row-group pitch, so matmuls zero-fill the
    rows between tiles (ts=42 -> 64, ts=21/16 -> 32)."""
    return 64 if spb in (2, 3) else 32


def _make_mask():
    """Per-spb one-hot walls [128, tsp, MAXNT]; dead partitions/rows zeroed."""
    masks = {}
    p = np.arange(128)
    for spb in sorted({s for _, s in CLASSES}):
        ts = 128 // spb
        used = ts * spb
        m = np.zeros((128, _tsp_of(spb), MAXNT), np.float16)
        pu = p[:used]
        m[pu, pu // spb, :] = 1.0
        masks[spb] = m
    return masks


def _build_program(tcounts, segs, nblocks, table_dt_name):
    import concourse.bacc as bacc
    import concourse.tile as tile
    import concourse.mybir as mybir
    from contextlib import ExitStack

    tdt = getattr(mybir.dt, table_dt_name)
    T = sum(tcounts)
    wc_cols = sum(CLASSES[ci][0] * tcounts[ci] for ci in range(len(CLASSES)))
    geo, out_rows = _class_geometry(tcounts, segs)

    nc = bacc.Bacc("TRN2", target_bir_lowering=False, debug=False)
    ctbl = {}
    for ci in range(len(CLASSES)):
        if tcounts[ci] == 0:
            continue
        nT = CLASSES[ci][0]
        ctbl[ci] = nc.dram_tensor(f"ctbl{ci}", [nblocks[ci], nT * C], tdt,
                                  kind="ExternalInput").ap()
    idxd = nc.dram_tensor("idx", [128, 8 * T], mybir.dt.int16,
                          kind="ExternalInput").ap()
    wcd = nc.dram_tensor("wc", [128, wc_cols], mybir.dt.float16,
                         kind="ExternalInput").ap()
    spbs = sorted({CLASSES[ci][1] for ci in range(len(CLASSES)) if tcounts[ci]})
    maskd = {}
    for spb in spbs:
        maskd[spb] = nc.dram_tensor(f"mask{spb}", [128, _tsp_of(spb), MAXNT],
                                    mybir.dt.float16, kind="ExternalInput").ap()
    outd = nc.dram_tensor("out", [out_rows, C], mybir.dt.float16,
                          kind="ExternalOutput").ap()

    with tile.TileContext(nc) as tc, ExitStack() as ctx:
        cpool = ctx.enter_context(tc.tile_pool(name="const", bufs=1))
        gpool = ctx.enter_context(tc.tile_pool(name="g", bufs=2))
        wpool = ctx.enter_context(tc.tile_pool(name="w", bufs=3))
        ppool = ctx.enter_context(tc.tile_pool(name="ps", bufs=8, space="PSUM"))
        opool = ctx.enter_context(tc.tile_pool(name="o", bufs=3))

        idx_sb = cpool.tile([128, 8 * T], mybir.dt.int16)
        nc.sync.dma_start(idx_sb[:], idxd[:])
        wc_sb = cpool.tile([128, wc_cols], mybir.dt.float16)
        mask_sb = {}
        for spb in maskd:
            mask_sb[spb] = cpool.tile([128, _tsp_of(spb), MAXNT],
                                      mybir.dt.float16,
                                      name=f"msk{spb}", tag=f"m{spb}")

        t_offs, c_offs = [], []
        to = co = 0
        for ci in range(len(CLASSES)):
            t_offs.append(to); c_offs.append(co)
            to += tcounts[ci]
            co += tcounts[ci] * CLASSES[ci][0]

        order = []
        mx = max((len(geo[ci]["batches"]) for ci in range(len(CLASSES))),
                 default=0)
        for bb in range(mx):
            for ci_ in range(len(CLASSES)):
                if bb < len(geo[ci_]["batches"]):
                    order.append((ci_, bb))

        def emit_gather(ci, bb):
            nT = CLASSES[ci][0]
            W = WBATCH[ci]
            t0, Wb, base, _stk0 = geo[ci]["batches"][bb]
            rows = min(1 << 15, nblocks[ci] - base)
            g = gpool.tile([128, W, nT * C], tdt, tag=f"g{ci}")
            i0 = (t_offs[ci] + t0) * 8
            nc.gpsimd.dma_gather(
                g[:, :Wb, :],
                ctbl[ci][base:base + rows, :],
                idx_sb[:, i0:i0 + Wb * 8],
                128 * Wb,
                128 * Wb,
                nT * C,
            )
            return g

        nc.sync.dma_start(wc_sb[:], wcd[:])
        for spb in maskd:
            nc.sync.dma_start(mask_sb[spb][:, :, :], maskd[spb][:, :, :])
        pending = {}
        for ci_, bb_ in order:
            if bb_ == 0:
                pending[(ci_, 0)] = emit_gather(ci_, 0)

        for ci, bb in order:
            nT, spb = CLASSES[ci]
            ts = _ts_of(ci)
            k = _k_of(ci)
            pbs = _pb_step(ci)
            msk = mask_sb[spb]
            t0, Wb, base, stk0 = geo[ci]["batches"][bb]
            c_off = c_offs[ci]
            g = pending.pop((ci, bb), None)
            if g is None:
                g = emit_gather(ci, bb)
            nstk = (Wb + k - 1) // k
            W = WBATCH[ci]
            tsp = _tsp_of(spb)
            ob = opool.tile([128, ((W + k - 1) // k) * C], mybir.dt.float16,
                            tag=f"ob{ci}")
            for s0 in range(0, nstk, 2):
                ncg = min(2, nstk - s0)
                psum = ppool.tile([128, 2 * C], mybir.dt.float32, tag="ps")
                rows_cg = []
                for cg in range(ncg):
                    s = s0 + cg
                    nrg = min(k, Wb - s * k)
                    rows_cg.append(nrg * pbs)
                    for rg in range(nrg):
                        wi = s * k + rg
                        tt = t0 + wi
                        pb = rg * pbs
                        wall = wpool.tile([128, tsp, nT], mybir.dt.float16,
                                          tag=f"wall{ci}")
                        co = c_off + tt * nT
                        nc.vector.tensor_tensor(
                            out=wall[:, :, :],
                            in0=wc_sb[:, co:co + nT].unsqueeze(1)
                                .broadcast_to([128, tsp, nT]),
                            in1=msk[:, :, :nT],
                            op=mybir.AluOpType.mult,
                        )
                        for j in range(nT):
                            nc.tensor.matmul(
                                psum[pb:pb + tsp, cg * C:(cg + 1) * C],
                                lhsT=wall[:, :, j],
                                rhs=g[:, wi, j * C:(j + 1) * C],
                                start=(j == 0),
                                stop=(j == nT - 1),
                                tile_position=(0, pb),
                            )
                if ncg == 2 and rows_cg[0] == rows_cg[1]:
                    nc.scalar.copy(ob[:rows_cg[0], s0 * C:(s0 + 2) * C],
                                   psum[:rows_cg[0], :2 * C])
                else:
                    for cg in range(ncg):
                        nc.scalar.copy(
                            ob[:rows_cg[cg], (s0 + cg) * C:(s0 + cg + 1) * C],
                            psum[:rows_cg[cg], cg * C:(cg + 1) * C])
                for cg in range(ncg):
                    if rows_cg[cg] < 128:
                        nc.vector.memset(
                            ob[rows_cg[cg]:, (s0 + cg) * C:(s0 + cg + 1) * C],
                            0.0)
            rbase = geo[ci]["r_off"] + stk0 * 128
            dview = outd[rbase:rbase + nstk * 128, :].rearrange(
                "(s p) c -> p s c", p=128)
            nc.sync.dma_start(
                dview, ob[:, :nstk * C].rearrange("p (s c) -> p s c", c=C))

    nc.compile()
    return nc


def prepare(feat0, feat1, feat2, feat3, boxes, img_ids, table_dt=TABLE_DT):
    np_dt = _np_table_dt(table_dt)
    tbl = _make_table((feat0, feat1, feat2, feat3), np_dt)
    cls, idxP, wcP = _host_precompute(boxes, img_ids)
    plans, tcounts, blkmaps, blkrefs, segs = _plan(cls, idxP)
    ctbls = _make_block_tables(tbl, blkmaps)
    nblocks = tuple(c.shape[0] for c in ctbls)
    geo, out_rows = _class_geometry(tcounts, segs)

    sig = (tcounts, tuple(tuple(s) for s in segs), nblocks, table_dt)
    if sig not in _PROGRAM_CACHE:
        _PROGRAM_CACHE[sig] = _build_program(tcounts, segs, nblocks, table_dt)
    nc = _PROGRAM_CACHE[sig]

    masks = _make_mask()
    in_maps = []
    slotmaps = []
    for c in range(NCORES):
        idx16, wc_arr, slotmap = _pack_core(c, plans, tcounts, blkrefs, wcP,
                                            geo, out_rows)
        im = {"idx": idx16, "wc": wc_arr}
        for ci in range(len(CLASSES)):
            if tcounts[ci] > 0:
                im[f"ctbl{ci}"] = ctbls[ci]
        for spb in sorted({CLASSES[ci][1] for ci in range(len(CLASSES))
                           if tcounts[ci]}):
            im[f"mask{spb}"] = masks[spb]
        in_maps.append(im)
        slotmaps.append(slotmap)
    return nc, in_maps, slotmaps


def assemble(results, slotmaps):
    final = np.zeros((N, C, NBIN), np.float32)
    for c in range(NCORES):
        out = results[c]["out"]
        sm = slotmaps[c]
        valid = sm >= 0
        ids = sm[valid]
        final[ids // NBIN, :, ids % NBIN] = out[valid].astype(np.float32)
    return final.reshape(N, C, OUT, OUT)


def kernel(feat0, feat1, feat2, feat3, boxes, img_ids):
    from concourse.bass_utils import run_bass_kernel_spmd

    nc, in_maps, slotmaps = prepare(feat0, feat1, feat2, feat3, boxes, img_ids)
    res = run_bass_kernel_spmd(nc, in_maps, list(range(NCORES)))
    return assemble(res.results, slotmaps)
